# revision 8
# baseline (speedup 1.0000x reference)
"""Causal self-attention (B=4, T=2048, C=1024, H=16) on 8 TRN2 NeuronCores.

Sharding: tensor-parallel over heads. Each core owns 2 heads:
  - c_attn: output columns (q,k,v dims) for its heads  -> [384, 1024] shard
  - attention: embarrassingly parallel over (B, local heads)
  - c_proj: input rows for its heads -> partial [B,T,C] output, summed on host

v2 layout/schedule (vs v1):
  - V computed token-major directly (lhsT = x^T tile, moving = Wv), so no PE
    transposes and no V bias on device: since softmax rows sum to 1, the V
    bias contributes exactly bv @ Wp.T to the output — folded in on host.
  - Self-pipelined batches: QKV/V GEMM work is staged by 512-token block;
    the stage-(s+1) units are emitted after attention of superblock s, so
    the in-order PE stream has independent matmuls to chew on while ACT
    grinds the (rate-limiting) exp stream. Batch b+1's first stage slots in
    after the last superblock of batch b.
  - y stored per 512-token superblock (y4 tiles); proj for superblock s is
    emitted after attention of superblock s+1, so proj never waits on the
    serial normalize chain.
  - Output partials stored/DMA'd as bf16 (halves out traffic; DMA engines
    are a serialized contended resource); summed in f32 on the host.
  - Batch-0 xt is loaded ts-block-major (one 3D DMA per 512-token block) so
    the cold start is ~4us instead of ~13; later batches prefetch per
    k-tile, two DMAs per superblock, to avoid monopolizing the DMA engines.
  - PSUM: 8 banks = qkv/v GEMM pool (2) + S tiles (3) + Y^T accum (2) +
    proj out (1). GPSIMD cannot read PSUM on TRN2: all PSUM->SBUF moves are
    on DVE (+ACT for half the proj stores); gpsimd only does SBUF-side work
    (ones-memset, reciprocal partition-broadcast).

Device layouts (host pre-transposed so matmul contraction sits on partitions):
  xt   [B, C, T]       x transposed; lhsT/rhs tiles [128 k, *]
  wqkv [128, 8, 384]   wqkv[p,k,n] = W_shard.T[k*128+p, n]
  bqkv [128, 3]        per-partition bias (cols: q, k, v; v unused on device)
  wp   [128, 1024]     wp[p,c] = W_proj[c, core*128+p]     (proj rhs)

Per-core pipeline per batch b:
  QK^T [128, 2, 2048] = W.T @ x.T + bias (DVE)
  V    [128 tok, 128 vdim] per token tile (direct GEMM) -> v2a [tok,jt,h,65]
       (65th column = 1.0 via memset; row 64 of Y^T = softmax denominator)
  per head h, per 512-col i-superblock, per 128-row j tile (causal only):
    S^T = K_j^T.T @ Q^T        [128 j, w i] PSUM   (w shrinks on diagonal)
    P^T = exp(S^T/64 + mask)   ACT -> SBUF, directly the PV rhs
    Y^T[65, 512] += V2aug_j.T @ P^T
  y4[i_sb] = Y^T[0:64] * bcast(approx 1/Y^T[64])
  proj: out[b, tok, :] = y4.T @ Wp^T -> bf16 SBUF stage -> DMA
Host: out = sum(partials) + b_proj + b_v @ W_proj.T
"""

import os
import sys

import numpy as np

os.environ.setdefault("MYCRO_LOCAL_CACHE", "1")
if "/opt/trn_rl_repo" not in sys.path:
    sys.path.insert(0, "/opt/trn_rl_repo")

B, T, C = 4, 2048, 1024
H, D = 16, 64
N_CORES = 8
HPC = H // N_CORES          # heads per core = 2
NL = HPC * D                # local width per q/k/v = 128
KT = C // 128               # 8 contraction tiles for QKV
NT = 3                      # q, k, v
SW = 512                    # i superblock width
NSB = T // SW               # 4 superblocks per batch
NJT = T // 128              # 16 j tiles per batch
NEG = -1.0e30

# matmul input dtype: bf16 (fastest), f32r (tf32-like), f32 (exact, 4x slow)
KDT = os.environ.get("KERNEL_DTYPE", "bf16")

_cache = {}
LAST_RESULT = None


def _np_mdt():
    if KDT == "bf16":
        import ml_dtypes
        return np.dtype(ml_dtypes.bfloat16)
    return np.dtype(np.float32)


def _build():
    import concourse.tile as tile
    from concourse import bacc, mybir

    dt = mybir.dt
    f32 = dt.float32
    mdt = {"bf16": dt.bfloat16, "f32r": dt.float32r, "f32": f32}[KDT]

    nc = bacc.Bacc("TRN2", target_bir_lowering=False, debug=False,
                   num_devices=N_CORES)

    xt = nc.dram_tensor("xt", [B, C, T], mdt, kind="ExternalInput").ap()
    wqkv = nc.dram_tensor("wqkv", [128, KT, NT * 128], mdt,
                          kind="ExternalInput").ap()
    bqkv = nc.dram_tensor("bqkv", [128, NT], f32, kind="ExternalInput").ap()
    wp = nc.dram_tensor("wp", [128, C], mdt, kind="ExternalInput").ap()
    # Output partials in bf16: halves the dominant out-DMA traffic; the 8
    # partials are summed in f32 on the host (adds ~1e-3 rel err).
    out = nc.dram_tensor("out", [B, T, C], dt.bfloat16,
                         kind="ExternalOutput").ap()

    # S^T layout: rows x = j (keys), cols y = i (queries); keep j <= i.
    # The causal mask is added in-place on the PSUM S tile by DVE (f32),
    # freeing the PE from the per-diag mask matmuls of v2.
    trit_np = np.where(np.arange(128)[:, None] <= np.arange(128)[None, :],
                       np.float32(0.0), np.float32(NEG))
    trit_dram = nc.inline_tensor(trit_np, name="tritmask").ap()

    Exp = mybir.ActivationFunctionType.Exp
    Ident = mybir.ActivationFunctionType.Identity

    _alt3 = [0]

    with tile.TileContext(nc) as tc:
        with (
            tc.tile_pool(name="consts", bufs=1) as consts,
            tc.tile_pool(name="xtp", bufs=2) as xtp,
            tc.tile_pool(name="qkp", bufs=2) as qkp,
            tc.tile_pool(name="v2p", bufs=2) as v2p,
            tc.tile_pool(name="y4p", bufs=6) as y4p,
            tc.tile_pool(name="ptp", bufs=8) as ptp,
            tc.tile_pool(name="stats", bufs=4) as stats,
            tc.tile_pool(name="stage", bufs=8) as stage,
            tc.tile_pool(name="rbp", bufs=3) as rbp,
            tc.tile_pool(name="mm_ps", bufs=2, space="PSUM") as mm_ps,
            tc.tile_pool(name="s_ps", bufs=3, space="PSUM") as s_ps,
            tc.tile_pool(name="yt_ps", bufs=2, space="PSUM") as yt_ps,
            tc.tile_pool(name="op_ps", bufs=1, space="PSUM") as op_ps,
        ):
            # HAM warm-up primer: dense dummy matmuls with no input deps so
            # the PE clock is at 2.4GHz by the time real work arrives.
            prime = consts.tile([128, SW], mdt if KDT != "f32r" else f32)
            nc.gpsimd.memset(prime[:], 0.25)
            for i in range(0 if KDT == "f32r" else 6):
                pps = s_ps.tile([128, SW], f32, tag="s")
                nc.tensor.matmul(pps[:], lhsT=prime[:, 0:128],
                                 rhs=prime[:], start=True, stop=True)

            # Prefire the Exp activation-table load (1.3us) into the cold
            # DMA window instead of paying it before the first real exp.
            actwarm = consts.tile([1, 1], f32)
            nc.scalar.activation(actwarm[:], prime[0:1, 0:1], Exp, scale=1.0)

            # wqkv arrives per-k interleaved with batch-0's first xt block
            # (emitted in emit_load_xt below) so the first QK matmul can
            # start ~1us in instead of waiting for the full weight DMA.
            wqkv_sb = consts.tile([128, KT, NT * 128], mdt)
            bias_sb = consts.tile([128, NT], f32)
            nc.sync.dma_start(bias_sb[:], bqkv[:])
            wp_sb = consts.tile([128, C], mdt)
            trit_sb = consts.tile([128, 128], f32)

            def emit_load_consts_rest():
                nc.sync.dma_start(wp_sb[:], wp[:])
                nc.sync.dma_start(trit_sb[:], trit_dram[:])

            def emit_load_xt(b, by_ts=False):
                """Returns (xt_sb, deferred) where deferred is a list of DMA
                closures the caller spreads out to avoid monopolizing the
                (serialized) DMA engines in one burst."""
                xt_sb = xtp.tile([128, KT, T], mdt, tag="xt")
                if by_ts:
                    # ts-column-major, one 3D DMA per 512-token block: the
                    # first block lands after ~3us so batch-0 QKV can start
                    # long before the full 4MB arrives. Cold batch only.
                    # Q/K weight columns land first; the V half follows the
                    # first xt block (V units run after the QK units).
                    nc.sync.dma_start(wqkv_sb[:, :, 0:256], wqkv[:, :, 0:256])
                    for ts in range(T // SW):
                        if ts == 0:
                            # first block in two halves so the QK k-chain
                            # can start on k0-3 while k4-7 is in flight
                            for kh in range(2):
                                nc.sync.dma_start(
                                    xt_sb[:, 4 * kh:4 * kh + 4, 0:SW],
                                    xt[b, 512 * kh:512 * kh + 512,
                                       0:SW].rearrange(
                                        "(k p) t -> p k t", p=128))
                            nc.sync.dma_start(wqkv_sb[:, :, 256:384],
                                              wqkv[:, :, 256:384])
                            emit_load_consts_rest()
                        else:
                            nc.sync.dma_start(
                                xt_sb[:, :, ts * SW:(ts + 1) * SW],
                                xt[b, :, ts * SW:(ts + 1) * SW].rearrange(
                                    "(k p) t -> p k t", p=128))
                    return xt_sb, []

                def dma(k):
                    def emit():
                        nc.sync.dma_start(xt_sb[:, k, :],
                                          xt[b, k * 128:(k + 1) * 128, :])
                    return emit
                return xt_sb, [dma(k) for k in range(KT)]

            def emit_qkv_units(xt_sb):
                """QK^T GEMM + direct token-major V GEMM, as a list of
                deferred emission units so the caller can interleave them
                into the previous batch's (ACT-bound) attention stream.

                Each unit is ~0.7-1.7us of independent PE work. Bias adds go
                to DVE and V copies to DVE/Pool so ACT stays exp-only."""
                qkt = qkp.tile([128, 2, T], mdt, tag="qkt")
                v2a = v2p.tile([128, NJT, HPC, 65], mdt, tag="v2a")
                nc.gpsimd.memset(v2a[:, :, :, 64:65], 1.0)
                units = []

                def qk_unit(n_t, ts):
                    def emit():
                        ps = mm_ps.tile([128, SW], f32, tag="mm")
                        for k in range(KT):
                            nc.tensor.matmul(
                                ps[:],
                                lhsT=wqkv_sb[:, k, n_t * 128:(n_t + 1) * 128],
                                rhs=xt_sb[:, k, ts * SW:(ts + 1) * SW],
                                start=(k == 0), stop=(k == KT - 1),
                            )
                        nc.vector.tensor_scalar_add(
                            qkt[:, n_t, ts * SW:(ts + 1) * SW], ps[:],
                            bias_sb[:, n_t:n_t + 1])
                    return emit

                def v_unit(mt):
                    def emit():
                        vps = mm_ps.tile([128, 128], f32, tag="mm")
                        for k in range(KT):
                            nc.tensor.matmul(
                                vps[:],
                                lhsT=xt_sb[:, k, mt * 128:(mt + 1) * 128],
                                rhs=wqkv_sb[:, k, 2 * 128:3 * 128],
                                start=(k == 0), stop=(k == KT - 1),
                            )
                        # one strided PSUM->SBUF cast covers both heads'
                        # 64-col V slices (dst stride jumps the ones column)
                        nc.vector.tensor_copy(
                            v2a[:, mt, :, 0:64],
                            vps[:, :].rearrange("p (h d) -> p h d", h=2))
                    return emit

                for n_t in range(2):
                    for ts in range(T // SW):
                        units.append(qk_unit(n_t, ts))
                for mt in range(NJT):
                    units.append(v_unit(mt))
                return qkt, v2a, units

            def emit_attn(qkt, v2a, i_sb, y4, fill=None):
                """Both heads x one 512-query superblock -> writes y4.

                v3: the two heads are paired PER j-tile. Their K_j lhsT
                tiles sit on disjoint SBUF partition halves (0-63 /
                64-127), so the two K=64 QK matmuls land in disjoint PE
                row-groups and execute CONCURRENTLY (the second LDWEIGHTS
                is pulled ahead over the first matmul) -- ~2x QK
                throughput vs the v2 head-serial stream. The causal mask
                moved from PE (ident.T @ trit accumulate) to a DVE in-PSUM
                add, and Y^T leaves PSUM right after the last PV so the
                paired accumulators only ever hold 2 PSUM banks.

                `fill` is a list of emission closures (proj tiles) popped
                two per j step to give the PE stream independent work."""
                njt = 4 * (i_sb + 1)
                q_ap = [qkt[h * 64:(h + 1) * 64, 0, :] for h in range(HPC)]
                k_ap = [qkt[h * 64:(h + 1) * 64, 1, :] for h in range(HPC)]
                yts = [yt_ps.tile([65, SW], f32, tag="yt", name=f"yt{h}")
                       for h in range(HPC)]
                for j_t in range(njt):
                    jtl = j_t - 4 * i_sb   # >=0 on the diagonal
                    diag = jtl >= 0
                    w = SW - jtl * 128 if diag else SW
                    i_lo = j_t * 128 if diag else i_sb * SW
                    sps = []
                    for h in range(HPC):
                        sp = s_ps.tile([128, SW], f32, tag="s")
                        sps.append(sp)
                        nc.tensor.matmul(
                            sp[:, :w],
                            lhsT=k_ap[h][:, j_t * 128:(j_t + 1) * 128],
                            rhs=q_ap[h][:, i_lo:i_lo + w],
                            start=True, stop=True,
                        )
                    if diag:
                        for h in range(HPC):
                            nc.vector.tensor_add(
                                sps[h][:, 0:128], sps[h][:, 0:128],
                                trit_sb[:])
                    for h in range(HPC):
                        pt = ptp.tile([128, SW], mdt, tag="pt")
                        nc.scalar.activation(
                            pt[:, :w], sps[h][:, :w], Exp, scale=1.0 / D)
                        nc.tensor.matmul(
                            yts[h][:, SW - w:SW],
                            lhsT=v2a[:, j_t, h, :],
                            rhs=pt[:, :w],
                            start=(j_t == 0), stop=(j_t == njt - 1),
                        )
                    for _ in range(2):
                        if fill:
                            fill.pop(0)()
                # normalize: y = yt[0:64] * bcast(approx 1/yt[64]).
                # yt is evacuated from PSUM immediately (banks freed for
                # the next superblock's paired accumulators); the denom
                # rows bounce to partition 0 (the approx-recip custom op
                # misreads PSUM/base-64 inputs on HW), both heads share
                # one recip + one partition-broadcast, and the final muls
                # run on the otherwise-idle GPSIMD from SBUF.
                ysb = stats.tile([64, HPC, SW], f32, tag="ysb")
                dnr = stats.tile([1, HPC * SW], f32, tag="dnr")
                for h in range(HPC):
                    nc.vector.tensor_copy(ysb[:, h, :], yts[h][0:64, :])
                    nc.vector.tensor_copy(
                        dnr[0:1, h * SW:(h + 1) * SW], yts[h][64:65, :])
                rcp = stats.tile([1, HPC * SW], f32, tag="rcp")
                nc.vector.reciprocal_approx_fast(out=rcp[:], in_=dnr[:])
                rb = rbp.tile([64, HPC * SW], f32, tag="rb")
                nc.gpsimd.partition_broadcast(rb[:], rcp[:])
                for h in range(HPC):
                    # DVE (not GPSIMD): the h=1 write crosses partition
                    # halves, which only DVE handles (proven in v2)
                    nc.vector.tensor_mul(
                        y4[h * 64:(h + 1) * 64, :], ysb[:, h, :],
                        rb[:, h * SW:(h + 1) * SW])

            def proj_units(b, i_sb, y4, pool=None, ptag="op"):
                """One closure per proj output tile (matmul+copy+DMA)."""
                pool = pool or op_ps
                # at the kernel tail nothing else issues DMAs or exps, so
                # the last proj's stores alternate SP/ACT issue queues to
                # halve the final drain (mid-kernel the ACT queue is poison:
                # a waiting DMA-issue head-of-line blocks the exp stream)
                tail = b == B - 1 and i_sb == NSB - 1
                def unit(mtl, c_h):
                    def emit():
                        op = pool.tile([128, SW], f32, tag=ptag, name="op")
                        nc.tensor.matmul(
                            op[:],
                            lhsT=y4[:, mtl * 128:(mtl + 1) * 128],
                            rhs=wp_sb[:, c_h * SW:(c_h + 1) * SW],
                            start=True, stop=True,
                        )
                        ost = stage.tile([128, SW], dt.bfloat16, tag="ost")
                        _alt3[0] ^= 1
                        if tail:
                            nc.vector.tensor_copy(ost[:], op[:])
                        elif _alt3[0]:
                            nc.vector.tensor_copy(ost[:], op[:])
                        else:
                            nc.scalar.copy(ost[:], op[:])
                        row = i_sb * SW + mtl * 128
                        eng = nc.scalar if tail and _alt3[0] else nc.sync
                        eng.dma_start(
                            out[b, row:row + 128,
                                c_h * SW:(c_h + 1) * SW], ost[:])
                    return emit
                return [unit(mtl, c_h) for mtl in range(SW // 128)
                        for c_h in range(C // SW)]

            # Self-pipelined schedule. Each batch's QKV/V GEMMs are staged by
            # 512-token block: attention on superblock s only needs QK blocks
            # <= s and V j-tiles < 4(s+1), so the stage-(s+1) GEMM units are
            # emitted right after attention of superblock s, giving the
            # in-order PE stream independent work while ACT grinds exps.
            def stage_units(units, s):
                """Units runnable once xt block s is resident: QK(nt, ts=s)
                and V(mt=4s..4s+3). units is ordered QK(nt0 ts0..3), QK(nt1
                ts0..3), V(mt0..15)."""
                return [units[s], units[4 + s]] + units[8 + 4 * s:12 + 4 * s]

            xt_sb, _ = emit_load_xt(0, by_ts=True)
            qkt, v2a, units = emit_qkv_units(xt_sb)
            for u in stage_units(units, 0):
                u()
            fill = []
            for b in range(B):
                nxt = None
                xt_dmas = []
                if b + 1 < B:
                    xt_nxt, xt_dmas = emit_load_xt(b + 1)
                    nxt = emit_qkv_units(xt_nxt)
                y4s = []
                for i_sb in range(NSB):
                    y4 = y4p.tile([128, SW], mdt, tag="y4")
                    y4s.append(y4)
                    emit_attn(qkt, v2a, i_sb, y4)
                    for u in fill:   # any fill not consumed by chain steps
                        u()
                    # the final batch's last proj tiles borrow the by-then
                    # idle mm_ps banks: with op_ps at 1 bank the tail proj
                    # would otherwise serialize mm -> copy -> mm
                    if b == B - 1 and i_sb >= NSB - 2:
                        fill = proj_units(b, i_sb, y4, pool=mm_ps, ptag="mm")
                    else:
                        fill = proj_units(b, i_sb, y4)
                    # next batch's xt prefetch, 2 k-tiles per superblock, so
                    # the loads never monopolize the DMA engines in a burst
                    for u in xt_dmas[2 * i_sb:2 * i_sb + 2]:
                        u()
                    if i_sb + 1 < NSB:
                        for u in stage_units(units, i_sb + 1):
                            u()
                    elif nxt is not None:
                        for u in stage_units(nxt[2], 0):
                            u()
                for u in fill:       # proj of the last superblock
                    u()
                fill = []
                if nxt is not None:
                    qkt, v2a, units = nxt

    nc.compile()
    return nc


def _get_nc():
    if "nc" not in _cache:
        _cache["nc"] = _build()
    return _cache["nc"]


def kernel(x, W_attn, b_attn, W_proj, b_proj):
    global LAST_RESULT
    from concourse.bass_utils import run_bass_kernel_spmd

    x = np.asarray(x, dtype=np.float32)
    W_attn = np.asarray(W_attn, dtype=np.float32)
    b_attn = np.asarray(b_attn, dtype=np.float32)
    W_proj = np.asarray(W_proj, dtype=np.float32)
    b_proj = np.asarray(b_proj, dtype=np.float32)

    nc = _get_nc()
    np_m = _np_mdt()

    xt = np.ascontiguousarray(x.transpose(0, 2, 1)).astype(np_m)
    in_maps = []
    for c in range(N_CORES):
        sl = slice(c * NL, (c + 1) * NL)
        w_shard = np.concatenate(
            [W_attn[sl], W_attn[C:2 * C][sl], W_attn[2 * C:][sl]], axis=0)
        # wqkv[p, k, n] = w_shard.T[k*128+p, n]
        wqkv = np.ascontiguousarray(
            w_shard.T.reshape(KT, 128, NT * 128).transpose(1, 0, 2)).astype(np_m)
        b_shard = np.concatenate(
            [b_attn[sl], b_attn[C:2 * C][sl], b_attn[2 * C:][sl]])
        bq = np.ascontiguousarray(b_shard.reshape(NT, 128).T)
        wp_c = np.ascontiguousarray(W_proj[:, sl].T).astype(np_m)
        in_maps.append({"xt": xt, "wqkv": wqkv, "bqkv": bq, "wp": wp_c})

    try:
        res = run_bass_kernel_spmd(nc, in_maps,
                                   core_ids=list(range(N_CORES)))
    except Exception:
        # one retry: transient NRT/device hiccups recover on re-run
        import time
        time.sleep(10)
        res = run_bass_kernel_spmd(nc, in_maps,
                                   core_ids=list(range(N_CORES)))
    LAST_RESULT = res

    acc = res.results[0]["out"].astype(np.float32)
    for c in range(1, N_CORES):
        acc = acc + res.results[c]["out"].astype(np.float32)
    # V bias folded out of the device kernel: softmax rows sum to 1, so the
    # missing bv contribution to the output is exactly bv @ W_proj.T.
    return acc + b_proj + b_attn[2 * C:] @ W_proj.T



# revision 24
# speedup vs baseline: 1.0000x; 1.0000x over previous
"""Causal self-attention (B=4, T=2048, C=1024, H=16) on 8 TRN2 NeuronCores.

Sharding: tensor-parallel over heads. Each core owns 2 heads:
  - c_attn: output columns (q,k,v dims) for its heads  -> [384, 1024] shard
  - attention: embarrassingly parallel over (B, local heads)
  - c_proj: input rows for its heads -> partial [B,T,C] output, summed on host

v2 layout/schedule (vs v1):
  - V computed token-major directly (lhsT = x^T tile, moving = Wv), so no PE
    transposes and no V bias on device: since softmax rows sum to 1, the V
    bias contributes exactly bv @ Wp.T to the output — folded in on host.
  - Self-pipelined batches: QKV/V GEMM work is staged by 512-token block;
    the stage-(s+1) units are emitted after attention of superblock s, so
    the in-order PE stream has independent matmuls to chew on while ACT
    grinds the (rate-limiting) exp stream. Batch b+1's first stage slots in
    after the last superblock of batch b.
  - y stored per 512-token superblock (y4 tiles); proj for superblock s is
    emitted after attention of superblock s+1, so proj never waits on the
    serial normalize chain.
  - Output partials stored/DMA'd as bf16 (halves out traffic; DMA engines
    are a serialized contended resource); summed in f32 on the host.
  - Batch-0 xt is loaded ts-block-major (one 3D DMA per 512-token block) so
    the cold start is ~4us instead of ~13; later batches prefetch per
    k-tile, two DMAs per superblock, to avoid monopolizing the DMA engines.
  - PSUM: 8 banks = qkv/v GEMM pool (2) + S tiles (3) + Y^T accum (2) +
    proj out (1). GPSIMD cannot read PSUM on TRN2: all PSUM->SBUF moves are
    on DVE (+ACT for half the proj stores); gpsimd only does SBUF-side work
    (ones-memset, reciprocal partition-broadcast).

Device layouts (host pre-transposed so matmul contraction sits on partitions):
  xt   [B, C, T]       x transposed; lhsT/rhs tiles [128 k, *]
  wqkv [128, 8, 384]   wqkv[p,k,n] = W_shard.T[k*128+p, n]
  bqkv [128, 3]        per-partition bias (cols: q, k, v; v unused on device)
  wp   [128, 1024]     wp[p,c] = W_proj[c, core*128+p]     (proj rhs)

Per-core pipeline per batch b:
  QK^T [128, 2, 2048] = W.T @ x.T + bias (DVE)
  V    [128 tok, 128 vdim] per token tile (direct GEMM) -> v2a [tok,jt,h,65]
       (65th column = 1.0 via memset; row 64 of Y^T = softmax denominator)
  per head h, per 512-col i-superblock, per 128-row j tile (causal only):
    S^T = K_j^T.T @ Q^T        [128 j, w i] PSUM   (w shrinks on diagonal)
    P^T = exp(S^T/64 + mask)   ACT -> SBUF, directly the PV rhs
    Y^T[65, 512] += V2aug_j.T @ P^T
  y4[i_sb] = Y^T[0:64] * bcast(approx 1/Y^T[64])
  proj: out[b, tok, :] = y4.T @ Wp^T -> bf16 SBUF stage -> DMA
Host: out = sum(partials) + b_proj + b_v @ W_proj.T
"""

import os
import sys

import numpy as np

os.environ.setdefault("MYCRO_LOCAL_CACHE", "1")
if "/opt/trn_rl_repo" not in sys.path:
    sys.path.insert(0, "/opt/trn_rl_repo")

B, T, C = 4, 2048, 1024
H, D = 16, 64
N_CORES = 8
HPC = H // N_CORES          # heads per core = 2
NL = HPC * D                # local width per q/k/v = 128
KT = C // 128               # 8 contraction tiles for QKV
NT = 3                      # q, k, v
SW = 512                    # i superblock width
NSB = T // SW               # 4 superblocks per batch
NJT = T // 128              # 16 j tiles per batch
NEG = -1.0e30

# matmul input dtype: bf16 (fastest), f32r (tf32-like), f32 (exact, 4x slow)
KDT = os.environ.get("KERNEL_DTYPE", "bf16")

_cache = {}
LAST_RESULT = None


def _np_mdt():
    if KDT == "bf16":
        import ml_dtypes
        return np.dtype(ml_dtypes.bfloat16)
    return np.dtype(np.float32)


def _build():
    import concourse.tile as tile
    from concourse import bacc, mybir

    dt = mybir.dt
    f32 = dt.float32
    mdt = {"bf16": dt.bfloat16, "f32r": dt.float32r, "f32": f32}[KDT]

    nc = bacc.Bacc("TRN2", target_bir_lowering=False, debug=False,
                   num_devices=N_CORES)

    xt = nc.dram_tensor("xt", [B, C, T], mdt, kind="ExternalInput").ap()
    wqkv = nc.dram_tensor("wqkv", [128, KT, NT * 128], mdt,
                          kind="ExternalInput").ap()
    bqkv = nc.dram_tensor("bqkv", [128, NT], f32, kind="ExternalInput").ap()
    wp = nc.dram_tensor("wp", [128, C], mdt, kind="ExternalInput").ap()
    # Output partials in bf16: halves the dominant out-DMA traffic; the 8
    # partials are summed in f32 on the host (adds ~1e-3 rel err).
    out = nc.dram_tensor("out", [B, T, C], dt.bfloat16,
                         kind="ExternalOutput").ap()

    # S^T layout: rows x = j (keys), cols y = i (queries); keep j <= i.
    # The causal mask is added in-place on the PSUM S pair-tile by DVE
    # (one add covers both heads), freeing the PE from the per-diag mask
    # matmuls of v2.
    trit_np = np.where(np.arange(128)[:, None] <= np.arange(128)[None, :],
                       np.float32(0.0), np.float32(NEG))
    trit2_np = np.concatenate([trit_np, trit_np], axis=1)
    trit2_dram = nc.inline_tensor(trit2_np, name="tritmask2").ap()

    Exp = mybir.ActivationFunctionType.Exp
    Ident = mybir.ActivationFunctionType.Identity

    _alt3 = [0]

    with tile.TileContext(nc) as tc:
        with (
            tc.tile_pool(name="consts", bufs=1) as consts,
            tc.tile_pool(name="xtp", bufs=2) as xtp,
            tc.tile_pool(name="qkp", bufs=2) as qkp,
            tc.tile_pool(name="v2p", bufs=2) as v2p,
            tc.tile_pool(name="y4p", bufs=6) as y4p,
            tc.tile_pool(name="ptp", bufs=4) as ptp,
            tc.tile_pool(name="stats", bufs=4) as stats,
            tc.tile_pool(name="stage", bufs=8) as stage,
            tc.tile_pool(name="rbp", bufs=3) as rbp,
            # PSUM: 8 banks = qkv/v/proj shared pool (2) + S pair-tiles
            # (2x2: both heads of one j step side by side) + Y^T accums (2)
            tc.tile_pool(name="mm_ps", bufs=2, space="PSUM") as mm_ps,
            tc.tile_pool(name="s_ps", bufs=2, space="PSUM") as s_ps,
            tc.tile_pool(name="yt_ps", bufs=2, space="PSUM") as yt_ps,
        ):
            # HAM warm-up primer: dense dummy matmuls with no input deps so
            # the PE clock is at 2.4GHz by the time real work arrives.
            prime = consts.tile([128, SW], mdt if KDT != "f32r" else f32)
            nc.gpsimd.memset(prime[:], 0.25)
            for i in range(0 if KDT == "f32r" else 6):
                pps = s_ps.tile([128, HPC, SW], f32, tag="s")
                nc.tensor.matmul(pps[:, 0, :], lhsT=prime[:, 0:128],
                                 rhs=prime[:], start=True, stop=True)

            # Prefire the Exp activation-table load (1.3us) into the cold
            # DMA window instead of paying it before the first real exp.
            actwarm = consts.tile([1, 1], f32)
            nc.scalar.activation(actwarm[:], prime[0:1, 0:1], Exp, scale=1.0)

            # wqkv arrives per-k interleaved with batch-0's first xt block
            # (emitted in emit_load_xt below) so the first QK matmul can
            # start ~1us in instead of waiting for the full weight DMA.
            wqkv_sb = consts.tile([128, KT, NT * 128], mdt)
            bias_sb = consts.tile([128, NT], f32)
            nc.sync.dma_start(bias_sb[:], bqkv[:])
            wp_sb = consts.tile([128, C], mdt)
            trit2_sb = consts.tile([128, HPC, 128], f32)

            def emit_load_consts_rest():
                nc.sync.dma_start(wp_sb[:], wp[:])
                nc.sync.dma_start(
                    trit2_sb[:],
                    trit2_dram[:].rearrange("p (h n) -> p h n", h=HPC))

            def emit_load_xt(b, by_ts=False):
                """Returns (xt_sb, deferred) where deferred is a list of DMA
                closures the caller spreads out to avoid monopolizing the
                (serialized) DMA engines in one burst."""
                xt_sb = xtp.tile([128, KT, T], mdt, tag="xt")
                if by_ts:
                    # ts-column-major, one 3D DMA per 512-token block: the
                    # first block lands after ~3us so batch-0 QKV can start
                    # long before the full 4MB arrives. Cold batch only.
                    # Q/K weight columns land first; the V half follows the
                    # first xt block (V units run after the QK units).
                    nc.sync.dma_start(wqkv_sb[:, :, 0:256], wqkv[:, :, 0:256])
                    for ts in range(T // SW):
                        if ts == 0:
                            # first block in two halves so the QK k-chain
                            # can start on k0-3 while k4-7 is in flight
                            for kh in range(2):
                                nc.sync.dma_start(
                                    xt_sb[:, 4 * kh:4 * kh + 4, 0:SW],
                                    xt[b, 512 * kh:512 * kh + 512,
                                       0:SW].rearrange(
                                        "(k p) t -> p k t", p=128))
                            nc.sync.dma_start(wqkv_sb[:, :, 256:384],
                                              wqkv[:, :, 256:384])
                            emit_load_consts_rest()
                        else:
                            nc.sync.dma_start(
                                xt_sb[:, :, ts * SW:(ts + 1) * SW],
                                xt[b, :, ts * SW:(ts + 1) * SW].rearrange(
                                    "(k p) t -> p k t", p=128))
                    return xt_sb, []

                def dma(k):
                    def emit():
                        nc.sync.dma_start(xt_sb[:, k, :],
                                          xt[b, k * 128:(k + 1) * 128, :])
                    return emit
                return xt_sb, [dma(k) for k in range(KT)]

            def emit_qkv_units(xt_sb):
                """QK^T GEMM + direct token-major V GEMM, as a list of
                deferred emission units so the caller can interleave them
                into the previous batch's (ACT-bound) attention stream.

                Each unit is ~0.7-1.7us of independent PE work. Bias adds go
                to DVE and V copies to DVE/Pool so ACT stays exp-only."""
                qkt = qkp.tile([128, 2, T], mdt, tag="qkt")
                v2a = v2p.tile([128, NJT, HPC, 65], mdt, tag="v2a")
                nc.gpsimd.memset(v2a[:, :, :, 64:65], 1.0)
                units = []

                def qk_unit(n_t, ts):
                    def emit():
                        ps = mm_ps.tile([128, SW], f32, tag="mm")
                        for k in range(KT):
                            nc.tensor.matmul(
                                ps[:],
                                lhsT=wqkv_sb[:, k, n_t * 128:(n_t + 1) * 128],
                                rhs=xt_sb[:, k, ts * SW:(ts + 1) * SW],
                                start=(k == 0), stop=(k == KT - 1),
                            )
                        nc.vector.tensor_scalar_add(
                            qkt[:, n_t, ts * SW:(ts + 1) * SW], ps[:],
                            bias_sb[:, n_t:n_t + 1])
                    return (2200, emit)   # ~8 x 270ns PE

                def v_unit(mt):
                    def emit():
                        vps = mm_ps.tile([128, 128], f32, tag="mm")
                        for k in range(KT):
                            nc.tensor.matmul(
                                vps[:],
                                lhsT=xt_sb[:, k, mt * 128:(mt + 1) * 128],
                                rhs=wqkv_sb[:, k, 2 * 128:3 * 128],
                                start=(k == 0), stop=(k == KT - 1),
                            )
                        # one strided PSUM->SBUF cast covers both heads'
                        # 64-col V slices (dst stride jumps the ones column)
                        nc.vector.tensor_copy(
                            v2a[:, mt, :, 0:64],
                            vps[:, :].rearrange("p (h d) -> p h d", h=2))
                    return (550, emit)    # ~8 x 56ns PE + cast latency

                for n_t in range(2):
                    for ts in range(T // SW):
                        units.append(qk_unit(n_t, ts))
                for mt in range(NJT):
                    units.append(v_unit(mt))
                return qkt, v2a, units

            def emit_attn(qkt, v2a, i_sb, y4, fill=None):
                """Both heads x one 512-query superblock -> writes y4.

                v3: the two heads are paired PER j-tile. Their K_j lhsT
                tiles sit on disjoint SBUF partition halves (0-63 /
                64-127), so the two K=64 QK matmuls land in disjoint PE
                row-groups and execute CONCURRENTLY (the second LDWEIGHTS
                is pulled ahead over the first matmul) -- ~2x QK
                throughput vs the v2 head-serial stream. The causal mask
                moved from PE (ident.T @ trit accumulate) to a DVE in-PSUM
                add, and Y^T leaves PSUM right after the last PV so the
                paired accumulators only ever hold 2 PSUM banks.

                `fill` is a list of (cost_ns, closure) emission units
                (proj tiles + the i_sb+1 QKV stage) spread evenly by cost
                across the j steps so the PE always has independent work
                but the exp-feeding QK chain is never starved."""
                njt = 4 * (i_sb + 1)
                per_step = (sum(c for c, _ in fill) / njt) if fill else 0.0
                budget = 0.0
                q_ap = [qkt[h * 64:(h + 1) * 64, 0, :] for h in range(HPC)]
                k_ap = [qkt[h * 64:(h + 1) * 64, 1, :] for h in range(HPC)]
                yts = [yt_ps.tile([65, SW], f32, tag="yt", name=f"yt{h}")
                       for h in range(HPC)]
                for j_t in range(njt):
                    jtl = j_t - 4 * i_sb   # >=0 on the diagonal
                    diag = jtl >= 0
                    w = SW - jtl * 128 if diag else SW
                    i_lo = j_t * 128 if diag else i_sb * SW
                    # one 2-bank PSUM pair-tile holds both heads' S: the
                    # QK matmuls release together (so they truly overlap
                    # in disjoint row-groups), the mask lands in one DVE
                    # add, and ONE exp covers both heads (halving the
                    # ~290ns fixed ACT cost per instruction)
                    sp = s_ps.tile([128, HPC, SW], f32, tag="s")
                    for h in range(HPC):
                        nc.tensor.matmul(
                            sp[:, h, :w],
                            lhsT=k_ap[h][:, j_t * 128:(j_t + 1) * 128],
                            rhs=q_ap[h][:, i_lo:i_lo + w],
                            start=True, stop=True,
                        )
                    if diag:
                        nc.vector.tensor_add(
                            sp[:, :, 0:128], sp[:, :, 0:128], trit2_sb[:])
                    pt = ptp.tile([128, HPC, SW], mdt, tag="pt")
                    nc.scalar.activation(
                        pt[:, :, :w], sp[:, :, :w], Exp, scale=1.0 / D)
                    for h in range(HPC):
                        nc.tensor.matmul(
                            yts[h][:, SW - w:SW],
                            lhsT=v2a[:, j_t, h, :],
                            rhs=pt[:, h, :w],
                            start=(j_t == 0), stop=(j_t == njt - 1),
                        )
                    budget += per_step
                    while fill and budget >= fill[0][0] - 1.0:
                        c, u = fill.pop(0)
                        u()
                        budget -= c
                # normalize: y = yt[0:64] * bcast(approx 1/yt[64]).
                # yt is evacuated from PSUM immediately (banks freed for
                # the next superblock's paired accumulators); the denom
                # rows bounce to partition 0 (the approx-recip custom op
                # misreads PSUM/base-64 inputs on HW), both heads share
                # one recip + one partition-broadcast, and the final muls
                # run on the otherwise-idle GPSIMD from SBUF.
                ysb = stats.tile([64, HPC, SW], f32, tag="ysb")
                dnr = stats.tile([1, HPC * SW], f32, tag="dnr")
                for h in range(HPC):
                    nc.vector.tensor_copy(ysb[:, h, :], yts[h][0:64, :])
                    nc.vector.tensor_copy(
                        dnr[0:1, h * SW:(h + 1) * SW], yts[h][64:65, :])
                rcp = stats.tile([1, HPC * SW], f32, tag="rcp")
                nc.vector.reciprocal_approx_fast(out=rcp[:], in_=dnr[:])
                rb = rbp.tile([64, HPC * SW], f32, tag="rb")
                nc.gpsimd.partition_broadcast(rb[:], rcp[:])
                for h in range(HPC):
                    # DVE (not GPSIMD): the h=1 write crosses partition
                    # halves, which only DVE handles (proven in v2)
                    nc.vector.tensor_mul(
                        y4[h * 64:(h + 1) * 64, :], ysb[:, h, :],
                        rb[:, h * SW:(h + 1) * SW])

            def proj_units(b, i_sb, y4):
                """One closure per proj output tile (matmul+copy+DMA).
                Output tiles share the mm_ps "mm" slots with the QKV/V
                chains (2 bufs pipeline any two fills)."""
                # at the kernel tail nothing else issues DMAs or exps, so
                # the last proj's stores alternate SP/ACT issue queues to
                # halve the final drain (mid-kernel the ACT queue is poison:
                # a waiting DMA-issue head-of-line blocks the exp stream)
                tail = b == B - 1 and i_sb == NSB - 1
                def unit(mtl, c_h):
                    def emit():
                        op = mm_ps.tile([128, SW], f32, tag="mm", name="op")
                        nc.tensor.matmul(
                            op[:],
                            lhsT=y4[:, mtl * 128:(mtl + 1) * 128],
                            rhs=wp_sb[:, c_h * SW:(c_h + 1) * SW],
                            start=True, stop=True,
                        )
                        ost = stage.tile([128, SW], dt.bfloat16, tag="ost")
                        _alt3[0] ^= 1
                        # mid-kernel the copy must stay OFF the ACT FIFO:
                        # a queued ACT copy head-of-line blocks the exp
                        # stream that paces the whole attention chain
                        nc.vector.tensor_copy(ost[:], op[:])
                        row = i_sb * SW + mtl * 128
                        eng = nc.scalar if tail and _alt3[0] else nc.sync
                        eng.dma_start(
                            out[b, row:row + 128,
                                c_h * SW:(c_h + 1) * SW], ost[:])
                    return (400, emit)    # one N=512 PE matmul + evac
                return [unit(mtl, c_h) for mtl in range(SW // 128)
                        for c_h in range(C // SW)]

            # Self-pipelined schedule. Each batch's QKV/V GEMMs are staged by
            # 512-token block: attention on superblock s only needs QK blocks
            # <= s and V j-tiles < 4(s+1), so the stage-(s+1) GEMM units are
            # emitted right after attention of superblock s, giving the
            # in-order PE stream independent work while ACT grinds exps.
            def stage_units(units, s):
                """Units runnable once xt block s is resident: QK(nt, ts=s)
                and V(mt=4s..4s+3). units is ordered QK(nt0 ts0..3), QK(nt1
                ts0..3), V(mt0..15)."""
                return [units[s], units[4 + s]] + units[8 + 4 * s:12 + 4 * s]

            # Emission plan: attention(i_sb) consumes, as paced fill work,
            # the previous superblock's proj units plus the QKV stage for
            # i_sb+1 (emitted one superblock early so the Q/K/V blocks it
            # needs are resident the moment its first j-step issues).
            xt_sb, _ = emit_load_xt(0, by_ts=True)
            qkt, v2a, units = emit_qkv_units(xt_sb)
            for _, u in stage_units(units, 0):
                u()
            fill = list(stage_units(units, 1))
            for b in range(B):
                nxt = None
                xt_dmas = []
                if b + 1 < B:
                    xt_nxt, xt_dmas = emit_load_xt(b + 1)
                    nxt = emit_qkv_units(xt_nxt)
                for i_sb in range(NSB):
                    y4 = y4p.tile([128, SW], mdt, tag="y4")
                    emit_attn(qkt, v2a, i_sb, y4, fill)
                    for _, u in fill:  # any fill not consumed by chain steps
                        u()
                    fill = proj_units(b, i_sb, y4)
                    # next batch's xt prefetch, 3 k-tiles per superblock:
                    # spread to not monopolize the DMA engines, but ALL
                    # emitted by i_sb=2 -- stage(next, 0) is consumed as
                    # fill during attention(i_sb=3) and contracts over
                    # every k-tile, so the DMAs must precede it
                    for u in xt_dmas[3 * i_sb:3 * i_sb + 3]:
                        u()
                    if i_sb + 2 < NSB:
                        fill += stage_units(units, i_sb + 2)
                    elif nxt is not None:
                        fill += stage_units(nxt[2], i_sb + 2 - NSB)
                if nxt is not None:
                    qkt, v2a, units = nxt
            for _, u in fill:        # proj of the last superblock
                u()

    nc.compile()
    return nc


def _get_nc():
    if "nc" not in _cache:
        _cache["nc"] = _build()
    return _cache["nc"]


def kernel(x, W_attn, b_attn, W_proj, b_proj):
    global LAST_RESULT
    from concourse.bass_utils import run_bass_kernel_spmd

    x = np.asarray(x, dtype=np.float32)
    W_attn = np.asarray(W_attn, dtype=np.float32)
    b_attn = np.asarray(b_attn, dtype=np.float32)
    W_proj = np.asarray(W_proj, dtype=np.float32)
    b_proj = np.asarray(b_proj, dtype=np.float32)

    nc = _get_nc()
    np_m = _np_mdt()

    xt = np.ascontiguousarray(x.transpose(0, 2, 1)).astype(np_m)
    in_maps = []
    for c in range(N_CORES):
        sl = slice(c * NL, (c + 1) * NL)
        w_shard = np.concatenate(
            [W_attn[sl], W_attn[C:2 * C][sl], W_attn[2 * C:][sl]], axis=0)
        # wqkv[p, k, n] = w_shard.T[k*128+p, n]
        wqkv = np.ascontiguousarray(
            w_shard.T.reshape(KT, 128, NT * 128).transpose(1, 0, 2)).astype(np_m)
        b_shard = np.concatenate(
            [b_attn[sl], b_attn[C:2 * C][sl], b_attn[2 * C:][sl]])
        bq = np.ascontiguousarray(b_shard.reshape(NT, 128).T)
        wp_c = np.ascontiguousarray(W_proj[:, sl].T).astype(np_m)
        in_maps.append({"xt": xt, "wqkv": wqkv, "bqkv": bq, "wp": wp_c})

    try:
        res = run_bass_kernel_spmd(nc, in_maps,
                                   core_ids=list(range(N_CORES)))
    except Exception:
        # one retry: transient NRT/device hiccups recover on re-run
        import time
        time.sleep(10)
        res = run_bass_kernel_spmd(nc, in_maps,
                                   core_ids=list(range(N_CORES)))
    LAST_RESULT = res

    acc = res.results[0]["out"].astype(np.float32)
    for c in range(1, N_CORES):
        acc = acc + res.results[c]["out"].astype(np.float32)
    # V bias folded out of the device kernel: softmax rows sum to 1, so the
    # missing bv contribution to the output is exactly bv @ W_proj.T.
    return acc + b_proj + b_attn[2 * C:] @ W_proj.T



# revision 32
# speedup vs baseline: 1.0027x; 1.0026x over previous
"""Causal self-attention (B=4, T=2048, C=1024, H=16) on 8 TRN2 NeuronCores.

Sharding: tensor-parallel over heads. Each core owns 2 heads:
  - c_attn: output columns (q,k,v dims) for its heads  -> [384, 1024] shard
  - attention: embarrassingly parallel over (B, local heads)
  - c_proj: input rows for its heads -> partial [B,T,C] output, summed on host

v2 layout/schedule (vs v1):
  - V computed token-major directly (lhsT = x^T tile, moving = Wv), so no PE
    transposes and no V bias on device: since softmax rows sum to 1, the V
    bias contributes exactly bv @ Wp.T to the output — folded in on host.
  - Self-pipelined batches: QKV/V GEMM work is staged by 512-token block;
    the stage-(s+1) units are emitted after attention of superblock s, so
    the in-order PE stream has independent matmuls to chew on while ACT
    grinds the (rate-limiting) exp stream. Batch b+1's first stage slots in
    after the last superblock of batch b.
  - y stored per 512-token superblock (y4 tiles); proj for superblock s is
    emitted after attention of superblock s+1, so proj never waits on the
    serial normalize chain.
  - Output partials stored/DMA'd as bf16 (halves out traffic; DMA engines
    are a serialized contended resource); summed in f32 on the host.
  - Batch-0 xt is loaded ts-block-major (one 3D DMA per 512-token block) so
    the cold start is ~4us instead of ~13; later batches prefetch per
    k-tile, two DMAs per superblock, to avoid monopolizing the DMA engines.
  - PSUM: 8 banks = qkv/v GEMM pool (2) + S tiles (3) + Y^T accum (2) +
    proj out (1). GPSIMD cannot read PSUM on TRN2: all PSUM->SBUF moves are
    on DVE (+ACT for half the proj stores); gpsimd only does SBUF-side work
    (ones-memset, reciprocal partition-broadcast).

Device layouts (host pre-transposed so matmul contraction sits on partitions):
  xt   [B, C, T]       x transposed; lhsT/rhs tiles [128 k, *]
  wqkv [128, 8, 384]   wqkv[p,k,n] = W_shard.T[k*128+p, n]
  bqkv [128, 3]        per-partition bias (cols: q, k, v; v unused on device)
  wp   [128, 1024]     wp[p,c] = W_proj[c, core*128+p]     (proj rhs)

Per-core pipeline per batch b:
  QK^T [128, 2, 2048] = W.T @ x.T + bias (DVE)
  V    [128 tok, 128 vdim] per token tile (direct GEMM) -> v2a [tok,jt,h,65]
       (65th column = 1.0 via memset; row 64 of Y^T = softmax denominator)
  per head h, per 512-col i-superblock, per 128-row j tile (causal only):
    S^T = K_j^T.T @ Q^T        [128 j, w i] PSUM   (w shrinks on diagonal)
    P^T = exp(S^T/64 + mask)   ACT -> SBUF, directly the PV rhs
    Y^T[65, 512] += V2aug_j.T @ P^T
  y4[i_sb] = Y^T[0:64] * bcast(approx 1/Y^T[64])
  proj: out[b, tok, :] = y4.T @ Wp^T -> bf16 SBUF stage -> DMA
Host: out = sum(partials) + b_proj + b_v @ W_proj.T
"""

import os
import sys

import numpy as np

os.environ.setdefault("MYCRO_LOCAL_CACHE", "1")
if "/opt/trn_rl_repo" not in sys.path:
    sys.path.insert(0, "/opt/trn_rl_repo")

B, T, C = 4, 2048, 1024
H, D = 16, 64
N_CORES = 8
HPC = H // N_CORES          # heads per core = 2
NL = HPC * D                # local width per q/k/v = 128
KT = C // 128               # 8 contraction tiles for QKV
NT = 3                      # q, k, v
SW = 512                    # i superblock width
NSB = T // SW               # 4 superblocks per batch
NJT = T // 128              # 16 j tiles per batch
NEG = -1.0e30

# matmul input dtype: bf16 (fastest), f32r (tf32-like), f32 (exact, 4x slow)
KDT = os.environ.get("KERNEL_DTYPE", "bf16")

# fp8 path: x, W_qkv, P, V in e4m3 (QK'^T itself stays bf16 on the q,k
# values for logit precision). Host pre-scales W_attn/b_attn by WSC so
# N(0, 1/C) weights sit in e4m3's normal range; the q,k scaling cancels
# in the exp scale and the v scaling in W_proj. End-to-end rel err
# measured 3e-3 vs the 2e-2 budget.
WSC = 32.0
EXPSC = 1.0 / (D * WSC * WSC)

_cache = {}
LAST_RESULT = None


def _np_mdt():
    if KDT == "bf16":
        import ml_dtypes
        return np.dtype(ml_dtypes.bfloat16)
    return np.dtype(np.float32)


def _build():
    import concourse.tile as tile
    from concourse import bacc, mybir

    dt = mybir.dt
    f32 = dt.float32
    e4 = dt.float8e4
    mdt = {"bf16": dt.bfloat16, "f32r": dt.float32r, "f32": f32}[KDT]
    DR = mybir.MatmulPerfMode.DoubleRow

    nc = bacc.Bacc("TRN2", target_bir_lowering=False, debug=False,
                   num_devices=N_CORES)

    xt = nc.dram_tensor("xt", [B, C, T], e4, kind="ExternalInput").ap()
    wqkv = nc.dram_tensor("wqkv", [128, KT, NT * 128], e4,
                          kind="ExternalInput").ap()
    bqkv = nc.dram_tensor("bqkv", [128, NT], f32, kind="ExternalInput").ap()
    wp = nc.dram_tensor("wp", [128, C], mdt, kind="ExternalInput").ap()
    # Output partials in bf16: halves the dominant out-DMA traffic; the 8
    # partials are summed in f32 on the host (adds ~1e-3 rel err).
    out = nc.dram_tensor("out", [B, T, C], dt.bfloat16,
                         kind="ExternalOutput").ap()

    # S^T layout: rows x = j (keys), cols y = i (queries); keep j <= i.
    # The causal mask is a PE accumulate (ident.T @ trit): it stays on the
    # in-order PE queue right behind the QK pair, which is lower-latency
    # on the exp critical path than a DVE hop.
    np_m = _np_mdt() if KDT == "bf16" else np.float32
    ident_np = np.eye(128).astype(np_m)
    trit_np = np.where(np.arange(128)[:, None] <= np.arange(128)[None, :],
                       np.float32(0.0), np.float32(NEG)).astype(np_m)
    ident_dram = nc.inline_tensor(ident_np, name="ident").ap()
    trit_dram = nc.inline_tensor(trit_np, name="tritmask").ap()

    Exp = mybir.ActivationFunctionType.Exp
    Ident = mybir.ActivationFunctionType.Identity

    _alt3 = [0]

    with tile.TileContext(nc) as tc:
        with (
            tc.tile_pool(name="consts", bufs=1) as consts,
            tc.tile_pool(name="xtp", bufs=2) as xtp,
            tc.tile_pool(name="qkp", bufs=2) as qkp,
            tc.tile_pool(name="v2p", bufs=2) as v2p,
            tc.tile_pool(name="y4p", bufs=6) as y4p,
            tc.tile_pool(name="ptp", bufs=4) as ptp,
            tc.tile_pool(name="stats", bufs=4) as stats,
            tc.tile_pool(name="stage", bufs=8) as stage,
            tc.tile_pool(name="rbp", bufs=3) as rbp,
            # PSUM: 8 banks = qkv/v/proj shared pool (2) + S pair-tiles
            # (2x2: both heads of one j step side by side) + Y^T accums (2)
            tc.tile_pool(name="mm_ps", bufs=2, space="PSUM") as mm_ps,
            tc.tile_pool(name="s_ps", bufs=2, space="PSUM") as s_ps,
            tc.tile_pool(name="yt_ps", bufs=2, space="PSUM") as yt_ps,
        ):
            # HAM warm-up primer: dense dummy matmuls with no input deps so
            # the PE clock is at 2.4GHz by the time real work arrives.
            prime = consts.tile([128, SW], mdt if KDT != "f32r" else f32)
            nc.gpsimd.memset(prime[:], 0.25)
            for i in range(0 if KDT == "f32r" else 6):
                pps = s_ps.tile([128, HPC, SW], f32, tag="s")
                nc.tensor.matmul(pps[:, 0, :], lhsT=prime[:, 0:128],
                                 rhs=prime[:], start=True, stop=True)

            # Prefire the Exp activation-table load (1.3us) into the cold
            # DMA window instead of paying it before the first real exp.
            actwarm = consts.tile([1, 1], f32)
            nc.scalar.activation(actwarm[:], prime[0:1, 0:1], Exp, scale=1.0)

            # wqkv arrives per-k interleaved with batch-0's first xt block
            # (emitted in emit_load_xt below) so the first QK matmul can
            # start ~1us in instead of waiting for the full weight DMA.
            wqkv_sb = consts.tile([128, KT, NT * 128], e4)
            bias_sb = consts.tile([128, NT], f32)
            nc.sync.dma_start(bias_sb[:], bqkv[:])
            wp_sb = consts.tile([128, C], mdt)
            ident_sb = consts.tile([128, 128], mdt)
            trit_sb = consts.tile([128, 128], mdt)

            def emit_load_consts_rest():
                nc.sync.dma_start(wp_sb[:], wp[:])
                nc.sync.dma_start(ident_sb[:], ident_dram[:].bitcast(mdt))
                nc.sync.dma_start(trit_sb[:], trit_dram[:].bitcast(mdt))

            def emit_load_xt(b, by_ts=False):
                """Returns (xt_sb, deferred) where deferred is a list of DMA
                closures the caller spreads out to avoid monopolizing the
                (serialized) DMA engines in one burst."""
                xt_sb = xtp.tile([128, KT, T], e4, tag="xt")
                if by_ts:
                    # ts-column-major, one 3D DMA per 512-token block: the
                    # first block lands after ~3us so batch-0 QKV can start
                    # long before the full 4MB arrives. Cold batch only.
                    # Q/K weight columns land first; the V half follows the
                    # first xt block (V units run after the QK units).
                    nc.sync.dma_start(wqkv_sb[:, :, 0:256], wqkv[:, :, 0:256])
                    for ts in range(T // SW):
                        if ts == 0:
                            # first block in two halves so the QK k-chain
                            # can start on k0-3 while k4-7 is in flight
                            for kh in range(2):
                                nc.sync.dma_start(
                                    xt_sb[:, 4 * kh:4 * kh + 4, 0:SW],
                                    xt[b, 512 * kh:512 * kh + 512,
                                       0:SW].rearrange(
                                        "(k p) t -> p k t", p=128))
                            nc.sync.dma_start(wqkv_sb[:, :, 256:384],
                                              wqkv[:, :, 256:384])
                            emit_load_consts_rest()
                        else:
                            nc.sync.dma_start(
                                xt_sb[:, :, ts * SW:(ts + 1) * SW],
                                xt[b, :, ts * SW:(ts + 1) * SW].rearrange(
                                    "(k p) t -> p k t", p=128))
                    return xt_sb, []

                def dma(k):
                    def emit():
                        nc.sync.dma_start(xt_sb[:, k, :],
                                          xt[b, k * 128:(k + 1) * 128, :])
                    return emit
                return xt_sb, [dma(k) for k in range(KT)]

            def emit_qkv_units(xt_sb):
                """QK^T GEMM + direct token-major V GEMM, as a list of
                deferred emission units so the caller can interleave them
                into the previous batch's (ACT-bound) attention stream.

                Each unit is ~0.7-1.7us of independent PE work. Bias adds go
                to DVE and V copies to DVE/Pool so ACT stays exp-only."""
                qkt = qkp.tile([128, 2, T], mdt, tag="qkt")
                # V+ones in e4m3, padded to 80 so the DoubleRow pair AP's
                # middle-dim byte stride (2*80) is a multiple of 16
                v2a = v2p.tile([128, NJT, HPC, 80], e4, tag="v2a")
                nc.gpsimd.memset(v2a[:, :, :, 64:65], 1.0)
                units = []

                def qk_unit(n_t, ts):
                    def emit():
                        ps = mm_ps.tile([128, SW], f32, tag="mm")
                        for k in range(KT // 2):
                            # fp8 DoubleRow: k-tile PAIRS, 256-deep virtual
                            # contraction, half the streaming cycles
                            nc.tensor.matmul(
                                ps[:],
                                lhsT=wqkv_sb[:, 2 * k:2 * k + 2,
                                             n_t * 128:(n_t + 1) * 128],
                                rhs=xt_sb[:, 2 * k:2 * k + 2,
                                          ts * SW:(ts + 1) * SW],
                                start=(k == 0), stop=(k == KT // 2 - 1),
                                perf_mode=DR,
                            )
                        nc.vector.tensor_scalar_add(
                            qkt[:, n_t, ts * SW:(ts + 1) * SW], ps[:],
                            bias_sb[:, n_t:n_t + 1])
                    return (1300, emit)   # ~4 x (DR-LDW + 107ns) PE

                def v_unit(mt):
                    def emit():
                        vps = mm_ps.tile([128, 128], f32, tag="mm")
                        for k in range(KT):
                            nc.tensor.matmul(
                                vps[:],
                                lhsT=xt_sb[:, k, mt * 128:(mt + 1) * 128],
                                rhs=wqkv_sb[:, k, 2 * 128:3 * 128],
                                start=(k == 0), stop=(k == KT - 1),
                            )
                        # one strided PSUM->SBUF cast covers both heads'
                        # 64-col V slices (dst stride jumps the ones column)
                        nc.vector.tensor_copy(
                            v2a[:, mt, :, 0:64],
                            vps[:, :].rearrange("p (h d) -> p h d", h=2))
                    return (550, emit)    # ~8 x 56ns PE + cast latency

                for n_t in range(2):
                    for ts in range(T // SW):
                        units.append(qk_unit(n_t, ts))
                for mt in range(NJT):
                    units.append(v_unit(mt))
                return qkt, v2a, units

            def emit_attn(qkt, v2a, i_sb, y4, fill=None):
                """Both heads x one 512-query superblock -> writes y4.

                v3: the two heads are paired PER j-tile. Their K_j lhsT
                tiles sit on disjoint SBUF partition halves (0-63 /
                64-127), so the two K=64 QK matmuls land in disjoint PE
                row-groups and execute CONCURRENTLY (the second LDWEIGHTS
                is pulled ahead over the first matmul) -- ~2x QK
                throughput vs the v2 head-serial stream. The causal mask
                moved from PE (ident.T @ trit accumulate) to a DVE in-PSUM
                add, and Y^T leaves PSUM right after the last PV so the
                paired accumulators only ever hold 2 PSUM banks.

                `fill` is a list of (cost_ns, closure) emission units
                (proj tiles + the i_sb+1 QKV stage) spread evenly by cost
                across the j steps so the PE always has independent work
                but the exp-feeding QK chain is never starved."""
                njt = 4 * (i_sb + 1)
                per_step = (sum(c for c, _ in fill) / njt) if fill else 0.0
                budget = 0.0
                q_ap = [qkt[h * 64:(h + 1) * 64, 0, :] for h in range(HPC)]
                k_ap = [qkt[h * 64:(h + 1) * 64, 1, :] for h in range(HPC)]
                yts = [yt_ps.tile([65, SW], f32, tag="yt", name=f"yt{h}")
                       for h in range(HPC)]
                for j_t in range(njt):
                    jtl = j_t - 4 * i_sb   # >=0 on the diagonal
                    diag = jtl >= 0
                    w = SW - jtl * 128 if diag else SW
                    i_lo = j_t * 128 if diag else i_sb * SW
                    # one 2-bank PSUM pair-tile holds both heads' S: the
                    # QK matmuls release together, so they truly overlap
                    # in disjoint PE row-groups (~2x QK throughput)
                    sp = s_ps.tile([128, HPC, SW], f32, tag="s")
                    for h in range(HPC):
                        nc.tensor.matmul(
                            sp[:, h, :w],
                            lhsT=k_ap[h][:, j_t * 128:(j_t + 1) * 128],
                            rhs=q_ap[h][:, i_lo:i_lo + w],
                            start=True, stop=not diag,
                        )
                    if diag:
                        # causal mask via PE accumulate: stays on the
                        # in-order PE queue right behind the QK pair --
                        # lower latency on the exp critical path than a
                        # DVE hop through a busy queue
                        for h in range(HPC):
                            nc.tensor.matmul(
                                sp[:, h, 0:128], lhsT=ident_sb[:],
                                rhs=trit_sb[:], start=False, stop=True)
                    # exps per head (not merged): ACT has slack, and the
                    # shorter instruction lets PV(h0) start while exp(h1)
                    # still streams. P is written in e4m3 into a tile
                    # PAIRED across two j steps, so the non-diag PV runs
                    # as one fp8 DoubleRow matmul per head with a 256-deep
                    # virtual contraction (half the streaming cycles).
                    if not diag:
                        if j_t % 2 == 0:
                            ptpair = ptp.tile([128, 2, HPC, SW], e4,
                                              tag="pt")
                        jp = j_t % 2
                        for h in range(HPC):
                            nc.scalar.activation(
                                ptpair[:, jp, h, :w], sp[:, h, :w], Exp,
                                scale=EXPSC)
                        if jp == 1:
                            for h in range(HPC):
                                nc.tensor.matmul(
                                    yts[h][:],
                                    lhsT=v2a[:, j_t - 1:j_t + 1, h, 0:65],
                                    rhs=ptpair[:, :, h, :],
                                    start=(j_t == 1), stop=False,
                                    perf_mode=DR,
                                )
                    else:
                        ptd = ptp.tile([128, 2, HPC, SW], e4, tag="pt")
                        for h in range(HPC):
                            nc.scalar.activation(
                                ptd[:, 0, h, :w], sp[:, h, :w], Exp,
                                scale=EXPSC)
                        for h in range(HPC):
                            nc.tensor.matmul(
                                yts[h][:, SW - w:SW],
                                lhsT=v2a[:, j_t, h, 0:65],
                                rhs=ptd[:, 0, h, :w],
                                start=(j_t == 0), stop=(j_t == njt - 1),
                            )
                    budget += per_step
                    while fill and budget >= fill[0][0] - 1.0:
                        c, u = fill.pop(0)
                        u()
                        budget -= c
                # normalize: y = yt[0:64] * bcast(approx 1/yt[64]).
                # yt is evacuated from PSUM immediately (banks freed for
                # the next superblock's paired accumulators); the denom
                # rows bounce to partition 0 (the approx-recip custom op
                # misreads PSUM/base-64 inputs on HW), both heads share
                # one recip + one partition-broadcast, and the final muls
                # run on the otherwise-idle GPSIMD from SBUF.
                ysb = stats.tile([64, HPC, SW], f32, tag="ysb")
                dnr = stats.tile([1, HPC * SW], f32, tag="dnr")
                for h in range(HPC):
                    nc.vector.tensor_copy(ysb[:, h, :], yts[h][0:64, :])
                    nc.vector.tensor_copy(
                        dnr[0:1, h * SW:(h + 1) * SW], yts[h][64:65, :])
                rcp = stats.tile([1, HPC * SW], f32, tag="rcp")
                nc.vector.reciprocal_approx_fast(out=rcp[:], in_=dnr[:])
                rb = rbp.tile([64, HPC * SW], f32, tag="rb")
                nc.gpsimd.partition_broadcast(rb[:], rcp[:])
                for h in range(HPC):
                    # DVE (not GPSIMD): the h=1 write crosses partition
                    # halves, which only DVE handles (proven in v2)
                    nc.vector.tensor_mul(
                        y4[h * 64:(h + 1) * 64, :], ysb[:, h, :],
                        rb[:, h * SW:(h + 1) * SW])

            def proj_units(b, i_sb, y4):
                """One closure per proj output tile (matmul+copy+DMA).
                Output tiles share the mm_ps "mm" slots with the QKV/V
                chains (2 bufs pipeline any two fills)."""
                # at the kernel tail nothing else issues DMAs or exps, so
                # the last proj's stores alternate SP/ACT issue queues to
                # halve the final drain (mid-kernel the ACT queue is poison:
                # a waiting DMA-issue head-of-line blocks the exp stream)
                tail = b == B - 1 and i_sb == NSB - 1
                def unit(mtl, c_h):
                    def emit():
                        op = mm_ps.tile([128, SW], f32, tag="mm", name="op")
                        nc.tensor.matmul(
                            op[:],
                            lhsT=y4[:, mtl * 128:(mtl + 1) * 128],
                            rhs=wp_sb[:, c_h * SW:(c_h + 1) * SW],
                            start=True, stop=True,
                        )
                        ost = stage.tile([128, SW], dt.bfloat16, tag="ost")
                        _alt3[0] ^= 1
                        # mid-kernel the copy must stay OFF the ACT FIFO:
                        # a queued ACT copy head-of-line blocks the exp
                        # stream that paces the whole attention chain
                        nc.vector.tensor_copy(ost[:], op[:])
                        row = i_sb * SW + mtl * 128
                        eng = nc.scalar if tail and _alt3[0] else nc.sync
                        eng.dma_start(
                            out[b, row:row + 128,
                                c_h * SW:(c_h + 1) * SW], ost[:])
                    return (400, emit)    # one N=512 PE matmul + evac
                return [unit(mtl, c_h) for mtl in range(SW // 128)
                        for c_h in range(C // SW)]

            # Self-pipelined schedule. Each batch's QKV/V GEMMs are staged by
            # 512-token block: attention on superblock s only needs QK blocks
            # <= s and V j-tiles < 4(s+1), so the stage-(s+1) GEMM units are
            # emitted right after attention of superblock s, giving the
            # in-order PE stream independent work while ACT grinds exps.
            def stage_units(units, s):
                """Units runnable once xt block s is resident: QK(nt, ts=s)
                and V(mt=4s..4s+3). units is ordered QK(nt0 ts0..3), QK(nt1
                ts0..3), V(mt0..15)."""
                return [units[s], units[4 + s]] + units[8 + 4 * s:12 + 4 * s]

            # Emission plan: attention(i_sb) consumes, as paced fill work,
            # the previous superblock's proj units plus the QKV stage for
            # i_sb+1 (emitted one superblock early so the Q/K/V blocks it
            # needs are resident the moment its first j-step issues).
            xt_sb, _ = emit_load_xt(0, by_ts=True)
            qkt, v2a, units = emit_qkv_units(xt_sb)
            for _, u in stage_units(units, 0):
                u()
            fill = list(stage_units(units, 1))
            for b in range(B):
                nxt = None
                xt_dmas = []
                if b + 1 < B:
                    xt_nxt, xt_dmas = emit_load_xt(b + 1)
                    nxt = emit_qkv_units(xt_nxt)
                for i_sb in range(NSB):
                    y4 = y4p.tile([128, SW], mdt, tag="y4")
                    emit_attn(qkt, v2a, i_sb, y4, fill)
                    for _, u in fill:  # any fill not consumed by chain steps
                        u()
                    fill = proj_units(b, i_sb, y4)
                    # next batch's xt prefetch, 3 k-tiles per superblock:
                    # spread to not monopolize the DMA engines, but ALL
                    # emitted by i_sb=2 -- stage(next, 0) is consumed as
                    # fill during attention(i_sb=3) and contracts over
                    # every k-tile, so the DMAs must precede it
                    for u in xt_dmas[3 * i_sb:3 * i_sb + 3]:
                        u()
                    if i_sb + 2 < NSB:
                        fill += stage_units(units, i_sb + 2)
                    elif nxt is not None:
                        fill += stage_units(nxt[2], i_sb + 2 - NSB)
                if nxt is not None:
                    qkt, v2a, units = nxt
            for _, u in fill:        # proj of the last superblock
                u()

    nc.compile()
    return nc


def _get_nc():
    if "nc" not in _cache:
        _cache["nc"] = _build()
    return _cache["nc"]


def kernel(x, W_attn, b_attn, W_proj, b_proj):
    global LAST_RESULT
    from concourse.bass_utils import run_bass_kernel_spmd

    x = np.asarray(x, dtype=np.float32)
    W_attn = np.asarray(W_attn, dtype=np.float32)
    b_attn = np.asarray(b_attn, dtype=np.float32)
    W_proj = np.asarray(W_proj, dtype=np.float32)
    b_proj = np.asarray(b_proj, dtype=np.float32)

    nc = _get_nc()
    np_m = _np_mdt()
    import ml_dtypes
    e4np = np.dtype(ml_dtypes.float8_e4m3)

    xt = np.ascontiguousarray(x.transpose(0, 2, 1))
    xt = np.clip(xt, -224.0, 224.0).astype(e4np)
    in_maps = []
    for c in range(N_CORES):
        sl = slice(c * NL, (c + 1) * NL)
        w_shard = np.concatenate(
            [W_attn[sl], W_attn[C:2 * C][sl], W_attn[2 * C:][sl]], axis=0)
        # wqkv[p, k, n] = w_shard.T[k*128+p, n]; x WSC so N(0, 1/C)
        # weights land in e4m3's normal range (compensated in EXPSC / wp)
        wqkv = np.ascontiguousarray(
            w_shard.T.reshape(KT, 128, NT * 128).transpose(1, 0, 2)) * WSC
        wqkv = np.clip(wqkv, -224.0, 224.0).astype(e4np)
        b_shard = np.concatenate(
            [b_attn[sl], b_attn[C:2 * C][sl], b_attn[2 * C:][sl]])
        bq = np.ascontiguousarray(b_shard.reshape(NT, 128).T) * WSC
        wp_c = np.ascontiguousarray(W_proj[:, sl].T / WSC).astype(np_m)
        in_maps.append({"xt": xt, "wqkv": wqkv, "bqkv": bq, "wp": wp_c})

    try:
        res = run_bass_kernel_spmd(nc, in_maps,
                                   core_ids=list(range(N_CORES)))
    except Exception:
        # one retry: transient NRT/device hiccups recover on re-run
        import time
        time.sleep(10)
        res = run_bass_kernel_spmd(nc, in_maps,
                                   core_ids=list(range(N_CORES)))
    LAST_RESULT = res

    acc = res.results[0]["out"].astype(np.float32)
    for c in range(1, N_CORES):
        acc = acc + res.results[c]["out"].astype(np.float32)
    # V bias folded out of the device kernel: softmax rows sum to 1, so the
    # missing bv contribution to the output is exactly bv @ W_proj.T.
    return acc + b_proj + b_attn[2 * C:] @ W_proj.T



# revision 38
# speedup vs baseline: 1.2863x; 1.2829x over previous
"""Causal self-attention (B=4, T=2048, C=1024, H=16) on 8 TRN2 NeuronCores.

Sharding: tensor-parallel over heads. Each core owns 2 heads:
  - c_attn: output columns (q,k,v dims) for its heads  -> [384, 1024] shard
  - attention: embarrassingly parallel over (B, local heads)
  - c_proj: input rows for its heads -> partial [B,T,C] output, summed on host

v2 layout/schedule (vs v1):
  - V computed token-major directly (lhsT = x^T tile, moving = Wv), so no PE
    transposes and no V bias on device: since softmax rows sum to 1, the V
    bias contributes exactly bv @ Wp.T to the output — folded in on host.
  - Self-pipelined batches: QKV/V GEMM work is staged by 512-token block;
    the stage-(s+1) units are emitted after attention of superblock s, so
    the in-order PE stream has independent matmuls to chew on while ACT
    grinds the (rate-limiting) exp stream. Batch b+1's first stage slots in
    after the last superblock of batch b.
  - y stored per 512-token superblock (y4 tiles); proj for superblock s is
    emitted after attention of superblock s+1, so proj never waits on the
    serial normalize chain.
  - Output partials stored/DMA'd as bf16 (halves out traffic; DMA engines
    are a serialized contended resource); summed in f32 on the host.
  - Batch-0 xt is loaded ts-block-major (one 3D DMA per 512-token block) so
    the cold start is ~4us instead of ~13; later batches prefetch per
    k-tile, two DMAs per superblock, to avoid monopolizing the DMA engines.
  - PSUM: 8 banks = qkv/v GEMM pool (2) + S tiles (3) + Y^T accum (2) +
    proj out (1). GPSIMD cannot read PSUM on TRN2: all PSUM->SBUF moves are
    on DVE (+ACT for half the proj stores); gpsimd only does SBUF-side work
    (ones-memset, reciprocal partition-broadcast).

Device layouts (host pre-transposed so matmul contraction sits on partitions):
  xt   [B, C, T]       x transposed; lhsT/rhs tiles [128 k, *]
  wqkv [128, 8, 384]   wqkv[p,k,n] = W_shard.T[k*128+p, n]
  bqkv [128, 3]        per-partition bias (cols: q, k, v; v unused on device)
  wp   [128, 1024]     wp[p,c] = W_proj[c, core*128+p]     (proj rhs)

Per-core pipeline per batch b:
  QK^T [128, 2, 2048] = W.T @ x.T + bias (DVE)
  V    [128 tok, 128 vdim] per token tile (direct GEMM) -> v2a [tok,jt,h,65]
       (65th column = 1.0 via memset; row 64 of Y^T = softmax denominator)
  per head h, per 512-col i-superblock, per 128-row j tile (causal only):
    S^T = K_j^T.T @ Q^T        [128 j, w i] PSUM   (w shrinks on diagonal)
    P^T = exp(S^T/64 + mask)   ACT -> SBUF, directly the PV rhs
    Y^T[65, 512] += V2aug_j.T @ P^T
  y4[i_sb] = Y^T[0:64] * bcast(approx 1/Y^T[64])
  proj: out[b, tok, :] = y4.T @ Wp^T -> bf16 SBUF stage -> DMA
Host: out = sum(partials) + b_proj + b_v @ W_proj.T
"""

import os
import sys

import numpy as np

os.environ.setdefault("MYCRO_LOCAL_CACHE", "1")
if "/opt/trn_rl_repo" not in sys.path:
    sys.path.insert(0, "/opt/trn_rl_repo")

B, T, C = 4, 2048, 1024
H, D = 16, 64
N_CORES = 8
HPC = H // N_CORES          # heads per core = 2
NL = HPC * D                # local width per q/k/v = 128
KT = C // 128               # 8 contraction tiles for QKV
NT = 3                      # q, k, v
SW = 512                    # i superblock width
NSB = T // SW               # 4 superblocks per batch
NJT = T // 128              # 16 j tiles per batch
NEG = -1.0e30

# matmul input dtype: bf16 (fastest), f32r (tf32-like), f32 (exact, 4x slow)
KDT = os.environ.get("KERNEL_DTYPE", "bf16")

# fp8 path: x, W_qkv, P, V in e4m3 (QK'^T itself stays bf16 on the q,k
# values for logit precision). Host pre-scales W_attn/b_attn by WSC so
# N(0, 1/C) weights sit in e4m3's normal range; the q,k scaling cancels
# in the exp scale and the v scaling in W_proj. End-to-end rel err
# measured 3e-3 vs the 2e-2 budget.
WSC = 32.0
EXPSC = 1.0 / (D * WSC * WSC)

# fill-work budget per attention j-step (ns of PE time): the attention
# phase is exp-throughput-bound at ~1.15us/step, of which ~0.6us is PE
# attention work; excess fill carries over to later (longer) superblocks
STEP_FILL = 900.0

_cache = {}
LAST_RESULT = None


def _np_mdt():
    if KDT == "bf16":
        import ml_dtypes
        return np.dtype(ml_dtypes.bfloat16)
    return np.dtype(np.float32)


def _build():
    import concourse.tile as tile
    from concourse import bacc, mybir

    dt = mybir.dt
    f32 = dt.float32
    e4 = dt.float8e4
    mdt = {"bf16": dt.bfloat16, "f32r": dt.float32r, "f32": f32}[KDT]
    DR = mybir.MatmulPerfMode.DoubleRow

    nc = bacc.Bacc("TRN2", target_bir_lowering=False, debug=False,
                   num_devices=N_CORES)

    xt = nc.dram_tensor("xt", [B, C, T], e4, kind="ExternalInput").ap()
    wqkv = nc.dram_tensor("wqkv", [128, KT, NT * 128], e4,
                          kind="ExternalInput").ap()
    bqkv = nc.dram_tensor("bqkv", [128, NT], f32, kind="ExternalInput").ap()
    wp = nc.dram_tensor("wp", [128, C], mdt, kind="ExternalInput").ap()
    # Output partials in bf16: halves the dominant out-DMA traffic; the 8
    # partials are summed in f32 on the host (adds ~1e-3 rel err).
    out = nc.dram_tensor("out", [B, T, C], dt.bfloat16,
                         kind="ExternalOutput").ap()

    # S^T layout: rows x = j (keys), cols y = i (queries); keep j <= i.
    # The causal mask is a PE accumulate (ident.T @ trit): it stays on the
    # in-order PE queue right behind the QK pair, which is lower-latency
    # on the exp critical path than a DVE hop.
    np_m = _np_mdt() if KDT == "bf16" else np.float32
    ident_np = np.eye(128).astype(np_m)
    trit_np = np.where(np.arange(128)[:, None] <= np.arange(128)[None, :],
                       np.float32(0.0), np.float32(NEG)).astype(np_m)
    ident_dram = nc.inline_tensor(ident_np, name="ident").ap()
    trit_dram = nc.inline_tensor(trit_np, name="tritmask").ap()

    Exp = mybir.ActivationFunctionType.Exp
    Ident = mybir.ActivationFunctionType.Identity

    _alt3 = [0]

    with tile.TileContext(nc) as tc:
        with (
            tc.tile_pool(name="consts", bufs=1) as consts,
            tc.tile_pool(name="xtp", bufs=2) as xtp,
            tc.tile_pool(name="qkp", bufs=2) as qkp,
            tc.tile_pool(name="v2p", bufs=2) as v2p,
            tc.tile_pool(name="y4p", bufs=6) as y4p,
            tc.tile_pool(name="ptp", bufs=4) as ptp,
            tc.tile_pool(name="stats", bufs=4) as stats,
            tc.tile_pool(name="stage", bufs=8) as stage,
            tc.tile_pool(name="rbp", bufs=3) as rbp,
            # PSUM: 8 banks = qkv/v/proj shared pool (2) + S pair-tiles
            # (2x2: both heads of one j step side by side) + Y^T accums (2)
            tc.tile_pool(name="mm_ps", bufs=2, space="PSUM") as mm_ps,
            tc.tile_pool(name="s_ps", bufs=2, space="PSUM") as s_ps,
            tc.tile_pool(name="yt_ps", bufs=2, space="PSUM") as yt_ps,
        ):
            # HAM warm-up primer: dense dummy matmuls with no input deps so
            # the PE clock is at 2.4GHz by the time real work arrives.
            prime = consts.tile([128, SW], mdt if KDT != "f32r" else f32)
            nc.gpsimd.memset(prime[:], 0.25)
            for i in range(0 if KDT == "f32r" else 6):
                pps = s_ps.tile([128, HPC, SW], f32, tag="s")
                nc.tensor.matmul(pps[:, 0, :], lhsT=prime[:, 0:128],
                                 rhs=prime[:], start=True, stop=True)

            # Prefire the Exp activation-table load (1.3us) into the cold
            # DMA window instead of paying it before the first real exp.
            actwarm = consts.tile([1, 1], f32)
            nc.scalar.activation(actwarm[:], prime[0:1, 0:1], Exp, scale=1.0)

            # wqkv arrives per-k interleaved with batch-0's first xt block
            # (emitted in emit_load_xt below) so the first QK matmul can
            # start ~1us in instead of waiting for the full weight DMA.
            wqkv_sb = consts.tile([128, KT, NT * 128], e4)
            bias_sb = consts.tile([128, NT], f32)
            nc.sync.dma_start(bias_sb[:], bqkv[:])
            wp_sb = consts.tile([128, C], mdt)
            ident_sb = consts.tile([128, 128], mdt)
            trit_sb = consts.tile([128, 128], mdt)

            def emit_load_consts_rest():
                nc.sync.dma_start(wp_sb[:], wp[:])
                nc.sync.dma_start(ident_sb[:], ident_dram[:].bitcast(mdt))
                nc.sync.dma_start(trit_sb[:], trit_dram[:].bitcast(mdt))

            def emit_load_xt(b, by_ts=False):
                """Returns (xt_sb, deferred) where deferred is a list of DMA
                closures the caller spreads out to avoid monopolizing the
                (serialized) DMA engines in one burst."""
                xt_sb = xtp.tile([128, KT, T], e4, tag="xt")
                if by_ts:
                    # ts-column-major, one 3D DMA per 512-token block: the
                    # first block lands after ~3us so batch-0 QKV can start
                    # long before the full 4MB arrives. Cold batch only.
                    # Q/K weight columns land first; the V half follows the
                    # first xt block (V units run after the QK units).
                    nc.sync.dma_start(wqkv_sb[:, :, 0:256], wqkv[:, :, 0:256])
                    for ts in range(T // SW):
                        if ts == 0:
                            # first block in two halves so the QK k-chain
                            # can start on k0-3 while k4-7 is in flight
                            for kh in range(2):
                                nc.sync.dma_start(
                                    xt_sb[:, 4 * kh:4 * kh + 4, 0:SW],
                                    xt[b, 512 * kh:512 * kh + 512,
                                       0:SW].rearrange(
                                        "(k p) t -> p k t", p=128))
                            nc.sync.dma_start(wqkv_sb[:, :, 256:384],
                                              wqkv[:, :, 256:384])
                            emit_load_consts_rest()
                        else:
                            nc.sync.dma_start(
                                xt_sb[:, :, ts * SW:(ts + 1) * SW],
                                xt[b, :, ts * SW:(ts + 1) * SW].rearrange(
                                    "(k p) t -> p k t", p=128))
                    return xt_sb, []

                def dma(k):
                    def emit():
                        nc.sync.dma_start(xt_sb[:, k, :],
                                          xt[b, k * 128:(k + 1) * 128, :])
                    return emit
                return xt_sb, [dma(k) for k in range(KT)]

            def emit_qkv_units(xt_sb):
                """QK^T GEMM + direct token-major V GEMM, as a list of
                deferred emission units so the caller can interleave them
                into the previous batch's (ACT-bound) attention stream.

                Each unit is ~0.7-1.7us of independent PE work. Bias adds go
                to DVE and V copies to DVE/Pool so ACT stays exp-only."""
                qkt = qkp.tile([128, 2, T], mdt, tag="qkt")
                # V+ones in e4m3, padded to 80 so the DoubleRow pair AP's
                # middle-dim byte stride (2*80) is a multiple of 16
                v2a = v2p.tile([128, NJT, HPC, 80], e4, tag="v2a")
                nc.gpsimd.memset(v2a[:, :, :, 64:65], 1.0)
                units = []

                def qk_unit(n_t, ts):
                    def emit():
                        ps = mm_ps.tile([128, SW], f32, tag="mm")
                        for k in range(KT // 2):
                            # fp8 DoubleRow: k-tile PAIRS, 256-deep virtual
                            # contraction, half the streaming cycles
                            nc.tensor.matmul(
                                ps[:],
                                lhsT=wqkv_sb[:, 2 * k:2 * k + 2,
                                             n_t * 128:(n_t + 1) * 128],
                                rhs=xt_sb[:, 2 * k:2 * k + 2,
                                          ts * SW:(ts + 1) * SW],
                                start=(k == 0), stop=(k == KT // 2 - 1),
                                perf_mode=DR,
                            )
                        nc.vector.tensor_scalar_add(
                            qkt[:, n_t, ts * SW:(ts + 1) * SW], ps[:],
                            bias_sb[:, n_t:n_t + 1])
                    return (1300, emit)   # ~4 x (DR-LDW + 107ns) PE

                def v_unit(mt):
                    def emit():
                        vps = mm_ps.tile([128, 128], f32, tag="mm")
                        for k in range(KT):
                            nc.tensor.matmul(
                                vps[:],
                                lhsT=xt_sb[:, k, mt * 128:(mt + 1) * 128],
                                rhs=wqkv_sb[:, k, 2 * 128:3 * 128],
                                start=(k == 0), stop=(k == KT - 1),
                            )
                        # one strided PSUM->SBUF cast covers both heads'
                        # 64-col V slices (dst stride jumps the ones column)
                        nc.vector.tensor_copy(
                            v2a[:, mt, :, 0:64],
                            vps[:, :].rearrange("p (h d) -> p h d", h=2))
                    return (550, emit)    # ~8 x 56ns PE + cast latency

                for n_t in range(2):
                    for ts in range(T // SW):
                        units.append(qk_unit(n_t, ts))
                for mt in range(NJT):
                    units.append(v_unit(mt))
                return qkt, v2a, units

            def emit_attn(qkt, v2a, i_sb, y4, stagef, fillq):
                """Both heads x one 512-query superblock -> writes y4.

                v3: the two heads are paired PER j-tile. Their K_j lhsT
                tiles sit on disjoint SBUF partition halves (0-63 /
                64-127), so the two K=64 QK matmuls land in disjoint PE
                row-groups and execute CONCURRENTLY (the second LDWEIGHTS
                is pulled ahead over the first matmul) -- ~2x QK
                throughput vs the v2 head-serial stream. The causal mask
                moved from PE (ident.T @ trit accumulate) to a DVE in-PSUM
                add, and Y^T leaves PSUM right after the last PV so the
                paired accumulators only ever hold 2 PSUM banks.

                `stagef` holds the i_sb+1 QKV stage units: they are paced
                first and force-drained at the end (hard deadline).
                `fillq` holds proj units: paced at a fixed per-step budget
                and CARRIED OVER across superblocks, so short superblocks
                (few j steps) don't choke on a burst of fill work -- that
                imbalance was measured as 2-11us exp-stream gaps at every
                superblock/batch boundary."""
                njt = 4 * (i_sb + 1)
                budget = 0.0
                q_ap = [qkt[h * 64:(h + 1) * 64, 0, :] for h in range(HPC)]
                k_ap = [qkt[h * 64:(h + 1) * 64, 1, :] for h in range(HPC)]
                yts = [yt_ps.tile([65, SW], f32, tag="yt", name=f"yt{h}")
                       for h in range(HPC)]
                for j_t in range(njt):
                    jtl = j_t - 4 * i_sb   # >=0 on the diagonal
                    diag = jtl >= 0
                    w = SW - jtl * 128 if diag else SW
                    i_lo = j_t * 128 if diag else i_sb * SW
                    # one 2-bank PSUM pair-tile holds both heads' S: the
                    # QK matmuls release together, so they truly overlap
                    # in disjoint PE row-groups (~2x QK throughput)
                    sp = s_ps.tile([128, HPC, SW], f32, tag="s")
                    for h in range(HPC):
                        nc.tensor.matmul(
                            sp[:, h, :w],
                            lhsT=k_ap[h][:, j_t * 128:(j_t + 1) * 128],
                            rhs=q_ap[h][:, i_lo:i_lo + w],
                            start=True, stop=not diag,
                        )
                    if diag:
                        # causal mask via PE accumulate: stays on the
                        # in-order PE queue right behind the QK pair --
                        # lower latency on the exp critical path than a
                        # DVE hop through a busy queue
                        for h in range(HPC):
                            nc.tensor.matmul(
                                sp[:, h, 0:128], lhsT=ident_sb[:],
                                rhs=trit_sb[:], start=False, stop=True)
                    # ONE merged exp covers both heads (the ~290ns fixed
                    # ACT cost is paid once per step, and the attention
                    # phase is exp-throughput-bound). P is written in
                    # e4m3 into a tile PAIRED across two j steps, so the
                    # non-diag PV runs as one fp8 DoubleRow matmul per
                    # head with a 256-deep virtual contraction (half the
                    # streaming cycles).
                    if not diag:
                        if j_t % 2 == 0:
                            ptpair = ptp.tile([128, 2, HPC, SW], e4,
                                              tag="pt")
                        jp = j_t % 2
                        nc.scalar.activation(
                            ptpair[:, jp, :, :w], sp[:, :, :w], Exp,
                            scale=EXPSC)
                        if jp == 1:
                            for h in range(HPC):
                                nc.tensor.matmul(
                                    yts[h][:],
                                    lhsT=v2a[:, j_t - 1:j_t + 1, h, 0:65],
                                    rhs=ptpair[:, :, h, :],
                                    start=(j_t == 1), stop=False,
                                    perf_mode=DR,
                                )
                    else:
                        ptd = ptp.tile([128, 2, HPC, SW], e4, tag="pt")
                        nc.scalar.activation(
                            ptd[:, 0, :, :w], sp[:, :, :w], Exp,
                            scale=EXPSC)
                        for h in range(HPC):
                            nc.tensor.matmul(
                                yts[h][:, SW - w:SW],
                                lhsT=v2a[:, j_t, h, 0:65],
                                rhs=ptd[:, 0, h, :w],
                                start=(j_t == 0), stop=(j_t == njt - 1),
                            )
                    budget += STEP_FILL
                    while stagef and budget >= stagef[0][0] - 1.0:
                        c, u = stagef.pop(0)
                        u()
                        budget -= c
                    while not stagef and fillq and budget >= fillq[0][0] - 1.0:
                        c, u = fillq.pop(0)
                        u()
                        budget -= c
                # stage units have a hard deadline (consumed by the next
                # superblock's attention): force-drain stragglers here.
                # fillq (proj) carries over instead.
                for c, u in stagef:
                    u()
                del stagef[:]
                # normalize: y = yt[0:64] * bcast(approx 1/yt[64]).
                # yt is evacuated from PSUM immediately (banks freed for
                # the next superblock's paired accumulators); the denom
                # rows bounce to partition 0 (the approx-recip custom op
                # misreads PSUM/base-64 inputs on HW), both heads share
                # one recip + one partition-broadcast, and the final muls
                # run on the otherwise-idle GPSIMD from SBUF.
                ysb = stats.tile([64, HPC, SW], f32, tag="ysb")
                dnr = stats.tile([1, HPC * SW], f32, tag="dnr")
                for h in range(HPC):
                    nc.vector.tensor_copy(ysb[:, h, :], yts[h][0:64, :])
                    nc.vector.tensor_copy(
                        dnr[0:1, h * SW:(h + 1) * SW], yts[h][64:65, :])
                rcp = stats.tile([1, HPC * SW], f32, tag="rcp")
                nc.vector.reciprocal_approx_fast(out=rcp[:], in_=dnr[:])
                rb = rbp.tile([64, HPC * SW], f32, tag="rb")
                nc.gpsimd.partition_broadcast(rb[:], rcp[:])
                for h in range(HPC):
                    # DVE (not GPSIMD): the h=1 write crosses partition
                    # halves, which only DVE handles (proven in v2)
                    nc.vector.tensor_mul(
                        y4[h * 64:(h + 1) * 64, :], ysb[:, h, :],
                        rb[:, h * SW:(h + 1) * SW])

            def proj_units(b, i_sb, y4):
                """One closure per proj output tile (matmul+copy+DMA).
                Output tiles share the mm_ps "mm" slots with the QKV/V
                chains (2 bufs pipeline any two fills)."""
                # at the kernel tail nothing else issues DMAs or exps, so
                # the last proj's stores alternate SP/ACT issue queues to
                # halve the final drain (mid-kernel the ACT queue is poison:
                # a waiting DMA-issue head-of-line blocks the exp stream)
                tail = b == B - 1 and i_sb == NSB - 1
                def unit(mtl, c_h):
                    def emit():
                        # the kernel-tail proj alternates into the by-then
                        # idle s_ps banks so MM/copy/DMA pipeline 4-wide
                        if tail and (2 * mtl + c_h) % 2:
                            op = s_ps.tile([128, SW], f32, tag="s",
                                           name="op")
                        else:
                            op = mm_ps.tile([128, SW], f32, tag="mm",
                                            name="op")
                        nc.tensor.matmul(
                            op[:],
                            lhsT=y4[:, mtl * 128:(mtl + 1) * 128],
                            rhs=wp_sb[:, c_h * SW:(c_h + 1) * SW],
                            start=True, stop=True,
                        )
                        ost = stage.tile([128, SW], dt.bfloat16, tag="ost")
                        _alt3[0] ^= 1
                        # mid-kernel the copy must stay OFF the ACT FIFO:
                        # a queued ACT copy head-of-line blocks the exp
                        # stream that paces the whole attention chain
                        nc.vector.tensor_copy(ost[:], op[:])
                        row = i_sb * SW + mtl * 128
                        eng = nc.scalar if tail and _alt3[0] else nc.sync
                        eng.dma_start(
                            out[b, row:row + 128,
                                c_h * SW:(c_h + 1) * SW], ost[:])
                    return (400, emit)    # one N=512 PE matmul + evac
                return [unit(mtl, c_h) for mtl in range(SW // 128)
                        for c_h in range(C // SW)]

            # Self-pipelined schedule. Each batch's QKV/V GEMMs are staged by
            # 512-token block: attention on superblock s only needs QK blocks
            # <= s and V j-tiles < 4(s+1), so the stage-(s+1) GEMM units are
            # emitted right after attention of superblock s, giving the
            # in-order PE stream independent work while ACT grinds exps.
            def stage_units(units, s):
                """Units runnable once xt block s is resident: QK(nt, ts=s)
                and V(mt=4s..4s+3). units is ordered QK(nt0 ts0..3), QK(nt1
                ts0..3), V(mt0..15)."""
                return [units[s], units[4 + s]] + units[8 + 4 * s:12 + 4 * s]

            # Emission plan: attention(i_sb) consumes, as paced fill work,
            # the previous superblock's proj units plus the QKV stage for
            # i_sb+1 (emitted one superblock early so the Q/K/V blocks it
            # needs are resident the moment its first j-step issues).
            xt_sb, _ = emit_load_xt(0, by_ts=True)
            qkt, v2a, units = emit_qkv_units(xt_sb)
            for _, u in stage_units(units, 0):
                u()
            stagef = list(stage_units(units, 1))
            fillq = []
            for b in range(B):
                nxt = None
                xt_dmas = []
                if b + 1 < B:
                    xt_nxt, xt_dmas = emit_load_xt(b + 1)
                    nxt = emit_qkv_units(xt_nxt)
                for i_sb in range(NSB):
                    y4 = y4p.tile([128, SW], mdt, tag="y4")
                    emit_attn(qkt, v2a, i_sb, y4, stagef, fillq)
                    fillq += proj_units(b, i_sb, y4)
                    # next batch's xt prefetch, 3 k-tiles per superblock:
                    # spread to not monopolize the DMA engines, but ALL
                    # emitted by i_sb=2 -- stage(next, 0) is consumed as
                    # fill during attention(i_sb=3) and contracts over
                    # every k-tile, so the DMAs must precede it
                    for u in xt_dmas[3 * i_sb:3 * i_sb + 3]:
                        u()
                    if i_sb + 2 < NSB:
                        stagef = list(stage_units(units, i_sb + 2))
                    elif nxt is not None:
                        stagef = list(stage_units(nxt[2], i_sb + 2 - NSB))
                    else:
                        stagef = []
                if nxt is not None:
                    qkt, v2a, units = nxt
            for _, u in fillq:       # carried-over + final proj tiles
                u()

    nc.compile()
    return nc


def _get_nc():
    if "nc" not in _cache:
        _cache["nc"] = _build()
    return _cache["nc"]


def kernel(x, W_attn, b_attn, W_proj, b_proj):
    global LAST_RESULT
    from concourse.bass_utils import run_bass_kernel_spmd

    x = np.asarray(x, dtype=np.float32)
    W_attn = np.asarray(W_attn, dtype=np.float32)
    b_attn = np.asarray(b_attn, dtype=np.float32)
    W_proj = np.asarray(W_proj, dtype=np.float32)
    b_proj = np.asarray(b_proj, dtype=np.float32)

    nc = _get_nc()
    np_m = _np_mdt()
    import ml_dtypes
    e4np = np.dtype(ml_dtypes.float8_e4m3)

    xt = np.ascontiguousarray(x.transpose(0, 2, 1))
    xt = np.clip(xt, -224.0, 224.0).astype(e4np)
    in_maps = []
    for c in range(N_CORES):
        sl = slice(c * NL, (c + 1) * NL)
        w_shard = np.concatenate(
            [W_attn[sl], W_attn[C:2 * C][sl], W_attn[2 * C:][sl]], axis=0)
        # wqkv[p, k, n] = w_shard.T[k*128+p, n]; x WSC so N(0, 1/C)
        # weights land in e4m3's normal range (compensated in EXPSC / wp)
        wqkv = np.ascontiguousarray(
            w_shard.T.reshape(KT, 128, NT * 128).transpose(1, 0, 2)) * WSC
        wqkv = np.clip(wqkv, -224.0, 224.0).astype(e4np)
        b_shard = np.concatenate(
            [b_attn[sl], b_attn[C:2 * C][sl], b_attn[2 * C:][sl]])
        bq = np.ascontiguousarray(b_shard.reshape(NT, 128).T) * WSC
        wp_c = np.ascontiguousarray(W_proj[:, sl].T / WSC).astype(np_m)
        in_maps.append({"xt": xt, "wqkv": wqkv, "bqkv": bq, "wp": wp_c})

    try:
        res = run_bass_kernel_spmd(nc, in_maps,
                                   core_ids=list(range(N_CORES)))
    except Exception:
        # one retry: transient NRT/device hiccups recover on re-run
        import time
        time.sleep(10)
        res = run_bass_kernel_spmd(nc, in_maps,
                                   core_ids=list(range(N_CORES)))
    LAST_RESULT = res

    acc = res.results[0]["out"].astype(np.float32)
    for c in range(1, N_CORES):
        acc = acc + res.results[c]["out"].astype(np.float32)
    # V bias folded out of the device kernel: softmax rows sum to 1, so the
    # missing bv contribution to the output is exactly bv @ W_proj.T.
    return acc + b_proj + b_attn[2 * C:] @ W_proj.T

